# revision 6
# baseline (speedup 1.0000x reference)
"""FP16-pulse -> FP8(E4M3)-pulse converter as a Trainium2 Bass/Tile kernel. v8.

Input : fp16_pulse [4096, 4096, 16] f32 of 0/1 bits, [S, E4..E0, M9..M0] MSB first.
Output: [4096, 4096, 8] f32 of 0/1 bits, [S, E3..E0, M2..M0].

Sharding: pure data-parallel over the leading batch dim (4096 -> 8 x 512).

Cost-model-driven schedule (the exclusive DMA device is the bottleneck at
~559.2us busy/core; the goal is DMA idle ~= 0). All op/engine/dtype combos
below were validated on the neuronxcc device path (AluOpType.mod and
scalar_tensor_tensor-on-Pool are ISA-invalid; bitVec ops cannot cast):
  * DVE runs the value chain with fast-mode ops: dual-ALU tensor_scalar
    (4x mode) and tensor_tensor (2x); scalar_tensor_tensor (1x) only for
    the Horners / omp / the two-tensor subtracts.
  * Output bits via the fp16-exponent-pin bitcast trick: u = min(oe,15)+16
    puts oei in mantissa bits 6..9 of u's fp16 pattern; (AND, shift) int16
    tensor_scalar ops extract each bit; same for omi via w = omi+8.
    ACT cast-copies the int16 bit tiles into the strided f32 output planes.
  * Pool: cf/sb scale construction + range masks (tensor_scalar only) and
    the SWDGE load issue; ACT: sign plane + 7 bit-plane copies.
  * Head taper [64,128,256] + first four loads on the idle SP HWDGE fill
    the DMA while the first chains warm up.
  * Drain: two 320-value tiles (H) are computed early, their stores gated
    on a 1-element identity STT that reads the second-to-last tile's input,
    so those stores become eligible exactly in the final stretch and cover
    the final small tile's (F) compute chain. The compiler list-schedules
    independent instructions, so the gate must be a real data dependency.
"""

import numpy as np
from contextlib import ExitStack

import concourse.bass as bass
import concourse.bacc as bacc
import concourse.tile as tile
from concourse import mybir
from concourse.bass_utils import run_bass_kernel_spmd

F32 = mybir.dt.float32
F16 = mybir.dt.float16
I16 = mybir.dt.int16
OP = mybir.AluOpType
ACTF = mybir.ActivationFunctionType

P = 128
N_CORES = 8
B0, B1 = 4096, 4096
NBITS, OBITS = 16, 8
MAGIC = 12582912.0  # 1.5 * 2**23: adding+subtracting rounds fp32 to int (RNE)

VALS_PER_PART = (B0 // N_CORES) * B1 // P  # 16384
VPT_FULL = 512
NTILES_FULL = VALS_PER_PART // VPT_FULL    # 32

INTERIOR = 448  # keeps xt at 4 SBUF buffers


def _tile_plan(total: int):
    """Returns (sizes, held_idx)."""
    if total < 4096:
        return [total], []
    head = [64, 128, 256]
    hsz = 320
    nheld = 3
    tail = [256, 192, 128]
    interior = total - sum(head) - nheld * hsz - sum(tail)
    nfull, rem = divmod(interior, INTERIOR)
    sizes = (head + [hsz] * nheld + ([rem] if rem else [])
             + [INTERIOR] * nfull + tail)
    return sizes, [len(head) + i for i in range(nheld)]


def build_nc(ntiles: int, vpt: int) -> bass.Bass:
    nc = bacc.Bacc()
    total = ntiles * vpt
    x = nc.declare_dram_parameter("x", [P, total * NBITS], F32, isOutput=False)
    y = nc.declare_dram_parameter("y", [P, total * OBITS], F32, isOutput=True)

    with tile.TileContext(nc) as tc, ExitStack() as ctx:
        iop = ctx.enter_context(tc.tile_pool(name="io", bufs=2))
        tp = ctx.enter_context(tc.tile_pool(name="tmp", bufs=2))

        V, G, S = nc.vector, nc.gpsimd, nc.scalar

        sizes, held = _tile_plan(total)
        assert sum(sizes) == total
        offs = np.concatenate([[0], np.cumsum(sizes)]).tolist()
        n = len(sizes)
        yts = {}

        for t, tsz in enumerate(sizes):
            off = offs[t]
            xt = iop.tile([P, NBITS * tsz], F32, tag="x", name="xt", bufs=4)
            if t < 4:
                # SP HWDGE for the first loads: shorter setup than SWDGE,
                # SP is otherwise idle this early, and Pool's first ops
                # (which wait on DVE) can't delay the initial fill.
                nc.sync.dma_start(xt[:], x[:, off * NBITS:(off + tsz) * NBITS])
            else:
                G.dma_start(xt[:], x[:, off * NBITS:(off + tsz) * NBITS])
            xb = xt[:].rearrange("p (v b) -> p v b", b=NBITS)
            if t in held:
                yt = iop.tile([P, OBITS * tsz], F32, tag=f"yh{t}", name="yh", bufs=1)
            else:
                yt = iop.tile([P, OBITS * tsz], F32, tag="y", name="yt", bufs=2)
            yts[t] = yt
            yb = yt[:].rearrange("p (v b) -> p v b", b=OBITS)

            gated = {n - 3: held[:1], n - 2: held[1:2], n - 1: held[2:]}.get(t, [])
            for h in gated:
                # gate held stores on this tile's input arriving: a 1-element
                # identity rewrite (yh = x*0 + yh) on DVE makes the held
                # store data-dependent on this late load (real dependency -
                # the compiler list-schedules independent instructions).
                V.scalar_tensor_tensor(
                    yts[h][0:1, 0:1], xt[0:1, 0:1], 0.0,
                    yts[h][0:1, 0:1], OP.mult, OP.add)

            def ib(j):
                return xb[:, :, j]

            def ob(j):
                return yb[:, :, j]

            def vt(dt, tag, bufs=2):
                return tp.tile([P, tsz], dt, tag=tag, name=tag, bufs=bufs)

            def sc(dt=F16):
                # one rotating scratch tag for DVE-local short-lived temps
                return tp.tile([P, tsz], dt, tag="s", name="s", bufs=8)

            # ---- sign plane passthrough [ACT]
            S.activation(ob(0), ib(0), ACTF.Copy)

            # ---- e = exponent field (Horner over E4..E0) [DVE]
            ea, eb_ = vt(F16, "ea"), vt(F16, "eb")
            V.scalar_tensor_tensor(ea[:], ib(1), 2.0, ib(2), OP.mult, OP.add)
            V.scalar_tensor_tensor(eb_[:], ea[:], 2.0, ib(3), OP.mult, OP.add)
            V.scalar_tensor_tensor(ea[:], eb_[:], 2.0, ib(4), OP.mult, OP.add)
            e = vt(F16, "e", bufs=3)
            V.scalar_tensor_tensor(e[:], ea[:], 2.0, ib(5), OP.mult, OP.add)

            # ---- m = mantissa field (Horner over M9..M0) [DVE]
            ma, mb = vt(F16, "ma"), vt(F16, "mb")
            V.scalar_tensor_tensor(ma[:], ib(6), 2.0, ib(7), OP.mult, OP.add)
            cur, alt = ma, mb
            for j in range(8, NBITS):
                V.scalar_tensor_tensor(alt[:], cur[:], 2.0, ib(j), OP.mult, OP.add)
                cur, alt = alt, cur
            m = cur

            # ---- scale 2^-clip(16-e,7,11) via fp16 exponent-field bitcast
            # and range masks [Pool, v1-proven positions]
            cf = vt(F16, "cf")
            G.tensor_scalar(cf[:], e[:], 9.0, 5.0, OP.min, OP.max)
            sb = vt(I16, "sb", bufs=3)
            G.tensor_scalar(sb[:], cf[:], -1.0, 1024.0, OP.add, OP.mult)
            sf = sb[:].bitcast(F16)
            c1 = vt(F16, "c1", bufs=3)
            G.tensor_scalar(c1[:], e[:], 5.0, None, OP.is_ge)
            t22 = vt(F16, "t22", bufs=3)
            G.tensor_scalar(t22[:], e[:], 22.0, None, OP.is_le)
            ovf6 = vt(F16, "ovf6", bufs=3)
            G.tensor_scalar(ovf6[:], e[:], 22.0, 6.0, OP.is_gt, OP.mult)

            # ---- x = m + 1024*(e<=8) ; y = x * scale ; q = RNE(y) [DVE]
            t8m = sc()
            V.tensor_scalar(t8m[:], e[:], 8.0, 1024.0, OP.is_le, OP.mult)
            xv = sc()
            V.tensor_add(xv[:], m[:], t8m[:])
            yv = sc()
            V.tensor_mul(yv[:], xv[:], sf)
            q = sc()
            V.tensor_scalar(q[:], yv[:], MAGIC, MAGIC, OP.add, OP.subtract)

            # ---- exponent assembly: oei = min(relu(e-8) + (q>=8), 15),
            # pinned into the fp16 mantissa of u = oei + 16 [DVE]
            carry = sc()
            V.tensor_scalar(carry[:], q[:], 8.0, None, OP.is_ge)
            d = sc()
            V.tensor_scalar(d[:], e[:], -8.0, 0.0, OP.add, OP.max)
            oe = sc()
            V.tensor_add(oe[:], d[:], carry[:])
            u = sc()
            V.tensor_scalar(u[:], oe[:], 15.0, 16.0, OP.min, OP.add)
            ub = u[:].bitcast(I16)

            # ---- mantissa assembly: omi = (q-8*carry)*(5<=e<=22) + 6*(e>22),
            # pinned into w = omi + 8 [DVE]
            omp = sc()
            V.scalar_tensor_tensor(omp[:], carry[:], -8.0, q[:], OP.mult, OP.add)
            nu2 = sc()
            V.tensor_mul(nu2[:], c1[:], t22[:])
            m2a = sc()
            V.tensor_mul(m2a[:], omp[:], nu2[:])
            w = sc()
            V.scalar_tensor_tensor(w[:], m2a[:], 8.0, ovf6[:], OP.add, OP.add)
            wb = w[:].bitcast(I16)

            # ---- bit extraction: (AND, >>) int16 ops [DVE], cast-copies
            # into the strided f32 planes [ACT]
            def bt():
                return tp.tile([P, tsz], I16, tag="bt", name="bt", bufs=8)

            for i, (src, mask, shr) in enumerate([
                (ub, 512, 9), (ub, 256, 8), (ub, 128, 7), (ub, 64, 6),
                (wb, 512, 9), (wb, 256, 8), (wb, 128, 7),
            ]):
                bit = bt()
                V.tensor_scalar(bit[:], src, mask, shr,
                                OP.bitwise_and, OP.logical_shift_right)
                S.activation(ob(1 + i), bit[:], ACTF.Copy)

            # rotating-buffer tiles must store in compute order (the next
            # tag-"y" tile reuses the buffer); held/final tiles store below.
            if (t < n - 1 and t not in held) or n == 1:
                nc.sync.dma_start(y[:, off * OBITS:(off + tsz) * OBITS], yt[:])

        if n > 1:
            # drain coverage: gated H stores, then the chain-dependent F store
            for t in held + [n - 1]:
                off, tsz = offs[t], sizes[t]
                nc.sync.dma_start(y[:, off * OBITS:(off + tsz) * OBITS], yts[t][:])
    nc.compile()
    return nc


_NC_CACHE: dict = {}


def _get_nc(ntiles: int, vpt: int) -> bass.Bass:
    key = (ntiles, vpt)
    if key not in _NC_CACHE:
        _NC_CACHE[key] = build_nc(ntiles, vpt)
    return _NC_CACHE[key]


def kernel(fp16_pulse: np.ndarray) -> np.ndarray:
    assert fp16_pulse.shape == (B0, B1, NBITS)
    in_dtype = fp16_pulse.dtype
    arr = np.ascontiguousarray(fp16_pulse, dtype=np.float32)
    rows = B0 // N_CORES
    in_maps = [
        {"x": arr[c * rows:(c + 1) * rows].reshape(P, VALS_PER_PART * NBITS)}
        for c in range(N_CORES)
    ]
    nc = _get_nc(NTILES_FULL, VPT_FULL)
    res = run_bass_kernel_spmd(nc, in_maps, list(range(N_CORES)))
    out = np.empty((B0, B1, OBITS), dtype=np.float32)
    for c in range(N_CORES):
        out[c * rows:(c + 1) * rows] = res.results[c]["y"].reshape(rows, B1, OBITS)
    return out.astype(in_dtype, copy=False)


# revision 7
# speedup vs baseline: 1.0016x; 1.0016x over previous
"""FP16-pulse -> FP8(E4M3)-pulse converter as a Trainium2 Bass/Tile kernel. v8.

Input : fp16_pulse [4096, 4096, 16] f32 of 0/1 bits, [S, E4..E0, M9..M0] MSB first.
Output: [4096, 4096, 8] f32 of 0/1 bits, [S, E3..E0, M2..M0].

Sharding: pure data-parallel over the leading batch dim (4096 -> 8 x 512).

Cost-model-driven schedule (the exclusive DMA device is the bottleneck at
~559.2us busy/core; the goal is DMA idle ~= 0). All op/engine/dtype combos
below were validated on the neuronxcc device path (AluOpType.mod and
scalar_tensor_tensor-on-Pool are ISA-invalid; bitVec ops cannot cast):
  * DVE runs the value chain with fast-mode ops: dual-ALU tensor_scalar
    (4x mode) and tensor_tensor (2x); scalar_tensor_tensor (1x) only for
    the Horners / omp / the two-tensor subtracts.
  * Output bits via the fp16-exponent-pin bitcast trick: u = min(oe,15)+16
    puts oei in mantissa bits 6..9 of u's fp16 pattern; (AND, shift) int16
    tensor_scalar ops extract each bit; same for omi via w = omi+8.
    ACT cast-copies the int16 bit tiles into the strided f32 output planes.
  * Pool: cf/sb scale construction + range masks (tensor_scalar only) and
    the SWDGE load issue; ACT: sign plane + 7 bit-plane copies.
  * Head taper [64,128,256] + first four loads on the idle SP HWDGE fill
    the DMA while the first chains warm up.
  * Drain: two 320-value tiles (H) are computed early, their stores gated
    on a 1-element identity STT that reads the second-to-last tile's input,
    so those stores become eligible exactly in the final stretch and cover
    the final small tile's (F) compute chain. The compiler list-schedules
    independent instructions, so the gate must be a real data dependency.
"""

import numpy as np
from contextlib import ExitStack

import concourse.bass as bass
import concourse.bacc as bacc
import concourse.tile as tile
from concourse import mybir
from concourse.bass_utils import run_bass_kernel_spmd

F32 = mybir.dt.float32
F16 = mybir.dt.float16
I16 = mybir.dt.int16
OP = mybir.AluOpType
ACTF = mybir.ActivationFunctionType

P = 128
N_CORES = 8
B0, B1 = 4096, 4096
NBITS, OBITS = 16, 8
MAGIC = 12582912.0  # 1.5 * 2**23: adding+subtracting rounds fp32 to int (RNE)

VALS_PER_PART = (B0 // N_CORES) * B1 // P  # 16384
VPT_FULL = 512
NTILES_FULL = VALS_PER_PART // VPT_FULL    # 32

INTERIOR = 432  # keeps xt at 4 SBUF buffers


def _tile_plan(total: int):
    """Returns (sizes, held_idx)."""
    if total < 4096:
        return [total], []
    head = [64, 128, 256]
    hsz = 320
    nheld = 3
    tail = [256, 192, 128]
    interior = total - sum(head) - nheld * hsz - sum(tail)
    nfull, rem = divmod(interior, INTERIOR)
    sizes = (head + [hsz] * nheld + ([rem] if rem else [])
             + [INTERIOR] * nfull + tail)
    return sizes, [len(head) + i for i in range(nheld)]


def build_nc(ntiles: int, vpt: int) -> bass.Bass:
    nc = bacc.Bacc()
    total = ntiles * vpt
    x = nc.declare_dram_parameter("x", [P, total * NBITS], F32, isOutput=False)
    y = nc.declare_dram_parameter("y", [P, total * OBITS], F32, isOutput=True)

    with tile.TileContext(nc) as tc, ExitStack() as ctx:
        iop = ctx.enter_context(tc.tile_pool(name="io", bufs=2))
        tp = ctx.enter_context(tc.tile_pool(name="tmp", bufs=2))

        V, G, S = nc.vector, nc.gpsimd, nc.scalar

        sizes, held = _tile_plan(total)
        assert sum(sizes) == total
        offs = np.concatenate([[0], np.cumsum(sizes)]).tolist()
        n = len(sizes)
        yts = {}

        for t, tsz in enumerate(sizes):
            off = offs[t]
            xt = iop.tile([P, NBITS * tsz], F32, tag="x", name="xt", bufs=4)
            if t < 4:
                # SP HWDGE for the first loads: shorter setup than SWDGE,
                # SP is otherwise idle this early, and Pool's first ops
                # (which wait on DVE) can't delay the initial fill.
                nc.sync.dma_start(xt[:], x[:, off * NBITS:(off + tsz) * NBITS])
            else:
                G.dma_start(xt[:], x[:, off * NBITS:(off + tsz) * NBITS])
            xb = xt[:].rearrange("p (v b) -> p v b", b=NBITS)
            if t in held:
                yt = iop.tile([P, OBITS * tsz], F32, tag=f"yh{t}", name="yh", bufs=1)
            else:
                yt = iop.tile([P, OBITS * tsz], F32, tag="y", name="yt", bufs=2)
            yts[t] = yt
            yb = yt[:].rearrange("p (v b) -> p v b", b=OBITS)

            gated = {n - 3: held[:1], n - 2: held[1:2], n - 1: held[2:]}.get(t, [])
            for h in gated:
                # gate held stores on this tile's input arriving: a 1-element
                # identity rewrite (yh = x*0 + yh) on DVE makes the held
                # store data-dependent on this late load (real dependency -
                # the compiler list-schedules independent instructions).
                V.scalar_tensor_tensor(
                    yts[h][0:1, 0:1], xt[0:1, 0:1], 0.0,
                    yts[h][0:1, 0:1], OP.mult, OP.add)

            def ib(j):
                return xb[:, :, j]

            def ob(j):
                return yb[:, :, j]

            def vt(dt, tag, bufs=2):
                return tp.tile([P, tsz], dt, tag=tag, name=tag, bufs=bufs)

            def sc(dt=F16):
                # one rotating scratch tag for DVE-local short-lived temps
                return tp.tile([P, tsz], dt, tag="s", name="s", bufs=8)

            # ---- sign plane passthrough [ACT]
            S.activation(ob(0), ib(0), ACTF.Copy)

            # ---- e = exponent field (Horner over E4..E0) [DVE]
            ea, eb_ = vt(F16, "ea"), vt(F16, "eb")
            V.scalar_tensor_tensor(ea[:], ib(1), 2.0, ib(2), OP.mult, OP.add)
            V.scalar_tensor_tensor(eb_[:], ea[:], 2.0, ib(3), OP.mult, OP.add)
            V.scalar_tensor_tensor(ea[:], eb_[:], 2.0, ib(4), OP.mult, OP.add)
            e = vt(F16, "e", bufs=3)
            V.scalar_tensor_tensor(e[:], ea[:], 2.0, ib(5), OP.mult, OP.add)

            # ---- m = mantissa field (Horner over M9..M0) [DVE]
            ma, mb = vt(F16, "ma"), vt(F16, "mb")
            V.scalar_tensor_tensor(ma[:], ib(6), 2.0, ib(7), OP.mult, OP.add)
            cur, alt = ma, mb
            for j in range(8, NBITS):
                V.scalar_tensor_tensor(alt[:], cur[:], 2.0, ib(j), OP.mult, OP.add)
                cur, alt = alt, cur
            m = cur

            # ---- scale 2^-clip(16-e,7,11) via fp16 exponent-field bitcast
            # and range masks [Pool, v1-proven positions]
            cf = vt(F16, "cf")
            G.tensor_scalar(cf[:], e[:], 9.0, 5.0, OP.min, OP.max)
            sb = vt(I16, "sb", bufs=3)
            G.tensor_scalar(sb[:], cf[:], -1.0, 1024.0, OP.add, OP.mult)
            sf = sb[:].bitcast(F16)
            c1 = vt(F16, "c1", bufs=3)
            G.tensor_scalar(c1[:], e[:], 5.0, None, OP.is_ge)
            t22 = vt(F16, "t22", bufs=3)
            G.tensor_scalar(t22[:], e[:], 22.0, None, OP.is_le)
            ovf6 = vt(F16, "ovf6", bufs=3)
            G.tensor_scalar(ovf6[:], e[:], 22.0, 6.0, OP.is_gt, OP.mult)

            # ---- x = m + 1024*(e<=8) ; y = x * scale ; q = RNE(y) [DVE]
            t8m = sc()
            V.tensor_scalar(t8m[:], e[:], 8.0, 1024.0, OP.is_le, OP.mult)
            xv = sc()
            V.tensor_add(xv[:], m[:], t8m[:])
            yv = sc()
            V.tensor_mul(yv[:], xv[:], sf)
            q = sc()
            V.tensor_scalar(q[:], yv[:], MAGIC, MAGIC, OP.add, OP.subtract)

            # ---- exponent assembly: oei = min(relu(e-8) + (q>=8), 15),
            # pinned into the fp16 mantissa of u = oei + 16 [DVE]
            carry = sc()
            V.tensor_scalar(carry[:], q[:], 8.0, None, OP.is_ge)
            d = sc()
            V.tensor_scalar(d[:], e[:], -8.0, 0.0, OP.add, OP.max)
            oe = sc()
            V.tensor_add(oe[:], d[:], carry[:])
            u = sc()
            V.tensor_scalar(u[:], oe[:], 15.0, 16.0, OP.min, OP.add)
            ub = u[:].bitcast(I16)

            # ---- mantissa assembly: omi = (q-8*carry)*(5<=e<=22) + 6*(e>22),
            # pinned into w = omi + 8 [DVE]
            omp = sc()
            V.scalar_tensor_tensor(omp[:], carry[:], -8.0, q[:], OP.mult, OP.add)
            nu2 = sc()
            V.tensor_mul(nu2[:], c1[:], t22[:])
            m2a = sc()
            V.tensor_mul(m2a[:], omp[:], nu2[:])
            w = sc()
            V.scalar_tensor_tensor(w[:], m2a[:], 8.0, ovf6[:], OP.add, OP.add)
            wb = w[:].bitcast(I16)

            # ---- bit extraction: (AND, >>) int16 ops [DVE], cast-copies
            # into the strided f32 planes [ACT]
            def bt():
                return tp.tile([P, tsz], I16, tag="bt", name="bt", bufs=8)

            for i, (src, mask, shr) in enumerate([
                (ub, 512, 9), (ub, 256, 8), (ub, 128, 7), (ub, 64, 6),
                (wb, 512, 9), (wb, 256, 8), (wb, 128, 7),
            ]):
                bit = bt()
                V.tensor_scalar(bit[:], src, mask, shr,
                                OP.bitwise_and, OP.logical_shift_right)
                S.activation(ob(1 + i), bit[:], ACTF.Copy)

            # rotating-buffer tiles must store in compute order (the next
            # tag-"y" tile reuses the buffer); held/final tiles store below.
            if (t < n - 1 and t not in held) or n == 1:
                nc.sync.dma_start(y[:, off * OBITS:(off + tsz) * OBITS], yt[:])

        if n > 1:
            # drain coverage: gated H stores, then the chain-dependent F store
            for t in held + [n - 1]:
                off, tsz = offs[t], sizes[t]
                nc.sync.dma_start(y[:, off * OBITS:(off + tsz) * OBITS], yts[t][:])
    nc.compile()
    return nc


_NC_CACHE: dict = {}


def _get_nc(ntiles: int, vpt: int) -> bass.Bass:
    key = (ntiles, vpt)
    if key not in _NC_CACHE:
        _NC_CACHE[key] = build_nc(ntiles, vpt)
    return _NC_CACHE[key]


def kernel(fp16_pulse: np.ndarray) -> np.ndarray:
    assert fp16_pulse.shape == (B0, B1, NBITS)
    in_dtype = fp16_pulse.dtype
    arr = np.ascontiguousarray(fp16_pulse, dtype=np.float32)
    rows = B0 // N_CORES
    in_maps = [
        {"x": arr[c * rows:(c + 1) * rows].reshape(P, VALS_PER_PART * NBITS)}
        for c in range(N_CORES)
    ]
    nc = _get_nc(NTILES_FULL, VPT_FULL)
    res = run_bass_kernel_spmd(nc, in_maps, list(range(N_CORES)))
    out = np.empty((B0, B1, OBITS), dtype=np.float32)
    for c in range(N_CORES):
        out[c * rows:(c + 1) * rows] = res.results[c]["y"].reshape(rows, B1, OBITS)
    return out.astype(in_dtype, copy=False)
